# revision 9
# baseline (speedup 1.0000x reference)
"""BEM (boundary evaluation module) Trainium2 kernel.

Strategy: shard the T=256 axis across 8 NeuronCores (32 own columns plus one
recomputed halo column on each side).  Each core runs the full pipeline on its
t-slice; the sampling GEMM and the Conv3d reduction are fused in SBUF so the
(B,C,N,T,W) intermediate never touches HBM.  GroupNorm statistics that span
the sharded axis are combined with three tiny HBM AllReduces.
"""

import os
import sys

import numpy as np

for _p in ("/opt/trn_rl_repo", "/root/.axon_site/_ro/trn_rl_repo"):
    if _p not in sys.path:
        sys.path.append(_p)

import ml_dtypes  # noqa: E402
import concourse.bass as bass  # noqa: E402
import concourse.bacc as bacc  # noqa: E402
import concourse.tile as tile  # noqa: E402
import concourse.mybir as mybir  # noqa: E402
from contextlib import ExitStack  # noqa: E402
from concourse.masks import make_identity  # noqa: E402

F32 = mybir.dt.float32
BF16 = mybir.dt.bfloat16
AF = mybir.ActivationFunctionType
ALU = mybir.AluOpType
BFNP = ml_dtypes.bfloat16

B = 2
DIM = 512
T = 256
H1 = 256
H3 = 512
H2 = 128
N = 32
W = 8
NCORES = 8
TOWN = T // NCORES          # 32 own t columns per core
TH = TOWN + 2               # with halo
COLS = TH * W               # 272
OWN_LO, OWN_HI = W, W + TOWN * W  # own column range inside the 272
EPS = 1e-5
NG = int(os.environ.get("KBEM_NG", "4"))  # mask n's per streamed group
NGRP = N // NG

# rows of the packed per-channel vector table
V_C1B = 0          # 2 rows (mt)
V_GN1G = 2         # 2
V_GN1B = 4         # 2
V_R3DB = 6         # 4 (ot)
V_GN3G = 10        # 4
V_GN3B = 14        # 4
V_R2DB = 18
V_GN2G = 19
V_GN2B = 20
V_S1B = 21
V_E1B = 22
V_SGNG = 23
V_SGNB = 24
V_EGNG = 25
V_EGNB = 26
V_S2B = 27         # s2 bias broadcast
V_E2B = 28
# batched per-instance tables (columns in instance order)
V_BG1G = 29        # 4: i = b*2+mt -> gn1_g[mt]
V_BG1B = 33
V_BG1C = 37        # c1_b[mt]
V_BG3G = 41        # 8: i = b*4+ot -> gn3_g[ot]
V_BG3B = 49
V_BG3C = 57        # r3d_b[ot]
V_BG2G = 65        # 2: i = b -> gn2_g
V_BG2B = 67
V_BG2C = 69        # r2d_b
V_BHG = 71         # 4: i = b*2+hd -> sgn_g/egn_g
V_BHB = 75
V_BHC = 79         # s1_b/e1_b
NVEC = 83

# wtail packing (bf16, [128, 23, 128]): r2d 0:4, s1 4:13, e1 13:22, s2 22
WT_R2D = 0
WT_S1 = 4
WT_E1 = 13
WT_S2 = 22

RG = [list(range(NCORES))]


def _build(iters=None):
    """Depth-4 software-pipelined emission: slot k emits F(k), T1(k-1),
    T2(k-2), T3(k-3) so the tensor queue never head-of-line blocks on the
    three GroupNorm AllReduce chains.

      F : loads, conv1+GN1(local)+transpose, sampling GEMM, Conv3d
          reduction, GN3 stats -> AllReduce#1
      T1: GN3 apply, r2d 1x1, GN2 stats -> AllReduce#2
      T2: GN2 apply -> f, halo scale, head 3x3 convs, head GN stats
          -> AllReduce#3
      T3: head GN apply, 1x1 sigmoid heads, output DMA
    """
    no_cc = bool(int(os.environ.get("KBEM_NO_CC", "0")))
    if iters is None:
        iters = int(os.environ.get("KBEM_ITERS", "1"))
    nc = bacc.Bacc("TRN2", target_bir_lowering=False, debug=False)

    xin = nc.declare_dram_parameter("x_in", [B, DIM, T], BF16, isOutput=False)
    maskin = nc.declare_dram_parameter("mask_in", [NGRP, 128, NG, 2, COLS], BF16, isOutput=False)
    c1w = nc.declare_dram_parameter("c1w", [128, 12, H1], BF16, isOutput=False)
    r3dw = nc.declare_dram_parameter("r3dw", [128, 64, H3], BF16, isOutput=False)
    wtail = nc.declare_dram_parameter("wtail", [128, 23, H2], BF16, isOutput=False)
    gmats = nc.declare_dram_parameter("gmats", [128, 56], F32, isOutput=False)
    emats = nc.declare_dram_parameter("emats", [96, 128], F32, isOutput=False)
    vecsd = nc.declare_dram_parameter("vecs", [NVEC, 128], F32, isOutput=False)
    hvd = nc.declare_dram_parameter("hv", [2], F32, isOutput=False)
    outd = nc.declare_dram_parameter("out", [B, 2, TOWN, W], F32, isOutput=True)

    with tile.TileContext(nc) as tc, ExitStack() as ctx:
        dram = ctx.enter_context(tc.tile_pool(name="dram", bufs=1, space="DRAM"))
        # version-k consts live until T3(k) at slot k+3 -> 4 rotating buffers
        constsS = ctx.enter_context(tc.tile_pool(name="constsS", bufs=4))
        constsW = ctx.enter_context(tc.tile_pool(name="constsW", bufs=4))
        # F-internal consts only need the F(k) / F(k+1) overlap
        constsC = ctx.enter_context(tc.tile_pool(name="constsC", bufs=2))
        bigres = ctx.enter_context(tc.tile_pool(name="bigres", bufs=1))
        r3ds = ctx.enter_context(tc.tile_pool(name="r3ds", bufs=int(os.environ.get("KBEM_R3DBUFS", "2"))))
        ver = ctx.enter_context(tc.tile_pool(name="ver", bufs=2))
        mstream = ctx.enter_context(tc.tile_pool(name="mstream", bufs=int(os.environ.get("KBEM_MBUFS", "2"))))
        small = ctx.enter_context(tc.tile_pool(name="small", bufs=16))
        psA = ctx.enter_context(tc.tile_pool(name="psA", bufs=int(os.environ.get("KBEM_PSA", "6")), space="PSUM"))
        psS = ctx.enter_context(tc.tile_pool(name="psS", bufs=int(os.environ.get("KBEM_PSS", "2")), space="PSUM"))

        # ---- GroupNorm helpers (version-bound via explicit args) ----
        def stats_from(src_ap, bias_ap, G, gdim, dst):
            """Scaled-group [mean, E[x^2]] of (src+bias) -> dst (gdim,2).
            G is pre-scaled by 1/(group_partitions * participating_cores) so
            the matmul (plus the later AllReduce) averages directly."""
            st6 = small.tile([128, 6], F32, name="st6", tag="st6")
            nc.vector.bn_stats(out=st6, in_=src_ap)
            mv = small.tile([128, 2], F32, name="mv", tag="mv")
            nc.vector.bn_aggr(out=mv, in_=st6)
            s12 = small.tile([128, 2], F32, name="s12", tag="s12")
            nc.vector.tensor_scalar_add(s12[:, 0:1], mv[:, 0:1], bias_ap)
            sq = small.tile([128, 1], F32, name="sq", tag="sq")
            nc.vector.tensor_mul(sq, s12[:, 0:1], s12[:, 0:1])
            nc.vector.tensor_add(s12[:, 1:2], mv[:, 1:2], sq)
            pg = psS.tile([gdim, 2], F32, name="pst", tag="pst")
            nc.tensor.matmul(pg, G[:, :], s12, start=True, stop=True)
            nc.vector.tensor_copy(dst, pg)

        def gn_finalize(stats_slice, gdim, rm_dst, epsT):
            var = small.tile([32, 1], F32, name="var", tag="var")[:gdim]
            sq = small.tile([32, 1], F32, name="sqg", tag="sqg")[:gdim]
            nc.vector.tensor_mul(sq, stats_slice[:, 0:1], stats_slice[:, 0:1])
            nc.vector.tensor_sub(var, stats_slice[:, 1:2], sq)
            nc.scalar.activation(out=var, in_=var, func=AF.Sqrt, bias=epsT[:gdim], scale=1.0)
            nc.vector.reciprocal(rm_dst[:, 0:1], var)
            nc.vector.tensor_copy(rm_dst[:, 1:2], stats_slice[:, 0:1])

        def gn_apply(E, gdim, rm_slice, gamma_ap, beta_ap, cbias_ap, src_ap, out_ap, func):
            pb = psS.tile([128, 2], F32, name="pst", tag="pst")
            nc.tensor.matmul(pb, E[:, :], rm_slice, start=True, stop=True)
            scale = small.tile([128, 1], F32, name="scale", tag="scale")
            nc.vector.tensor_mul(scale, pb[:, 0:1], gamma_ap)
            t1 = small.tile([128, 1], F32, name="t1", tag="t1")
            nc.vector.tensor_sub(t1, cbias_ap, pb[:, 1:2])
            t2 = small.tile([128, 1], F32, name="t2", tag="t2")
            nc.vector.tensor_mul(t2, t1, scale)
            bias = small.tile([128, 1], F32, name="bias", tag="bias")
            nc.vector.tensor_add(bias, t2, beta_ap)
            nc.scalar.activation(out=out_ap, in_=src_ap, func=func, bias=bias, scale=scale)

        def gn_batch(E, gdim, stg_view, ni, gG, gB, gC, srcs, outs, func, vec_sb, epsT):
            rm = small.tile([32, 8, 2], F32, name="rmb", tag="rmb")[:gdim, :ni, :]
            sq = small.tile([32, 8], F32, name="sqb", tag="sqb")[:gdim, :ni]
            var = small.tile([32, 8], F32, name="varb", tag="varb")[:gdim, :ni]
            nc.vector.tensor_mul(sq, stg_view[:, :, 0], stg_view[:, :, 0])
            nc.vector.tensor_sub(var, stg_view[:, :, 1], sq)
            nc.scalar.activation(out=var, in_=var, func=AF.Sqrt, bias=epsT[:gdim], scale=1.0)
            nc.vector.reciprocal(rm[:, :, 0], var)
            nc.vector.tensor_copy(rm[:, :, 1], stg_view[:, :, 0])
            pb = psS.tile([128, 8, 2], F32, name="pst", tag="pst")[:, :ni, :]
            nc.tensor.matmul(pb, E[:, :], rm, start=True, stop=True)
            scale = small.tile([128, 8], F32, name="scaleb", tag="scaleb")[:, :ni]
            bias = small.tile([128, 8], F32, name="biasb", tag="biasb")[:, :ni]
            t1 = small.tile([32, 8], F32, name="t1c", tag="t1c")[:, :ni] if False else small.tile([128, 8], F32, name="t1b", tag="t1b")[:, :ni]
            nc.vector.tensor_mul(scale, pb[:, :, 0], vec_sb[:, gG:gG + ni])
            nc.vector.tensor_sub(t1, vec_sb[:, gC:gC + ni], pb[:, :, 1])
            nc.vector.tensor_mul(t1, t1, scale)
            nc.vector.tensor_add(bias, t1, vec_sb[:, gB:gB + ni])
            for i in range(ni):
                nc.scalar.activation(out=outs[i], in_=srcs[i], func=func,
                                     bias=bias[:, i:i + 1], scale=scale[:, i:i + 1])

        def stage_F(k):
            S = {"k": k}
            # ---- loads ----
            x_sb = bigres.tile([128, 4, B, T + 2], BF16, name="x_sb", tag="x_sb")
            nc.vector.memset(x_sb[:, :, :, 0:1], 0.0)
            nc.vector.memset(x_sb[:, :, :, T + 1:T + 2], 0.0)
            for b in range(B):
                nc.sync.dma_start(
                    out=x_sb[:, :, b, 1:T + 1],
                    in_=bass.AP(tensor=xin, offset=b * DIM * T,
                                ap=[[T, 128], [128 * T, 4], [1, T]]))
            c1w_sb = constsC.tile([128, 12, H1], BF16, name="c1w_sb", tag="c1w_sb")
            nc.sync.dma_start(out=c1w_sb, in_=c1w[:, :, :])
            vec_sb = constsS.tile([128, NVEC], F32, name="vec_sb", tag="vec_sb")
            nc.sync.dma_start(out=vec_sb, in_=bass.AP(tensor=vecsd, offset=0, ap=[[1, 128], [128, NVEC]]))
            gm_sb = constsS.tile([128, 56], F32, name="gm_sb", tag="gm_sb")
            nc.sync.dma_start(out=gm_sb, in_=gmats[:, :])
            e8_sb = constsC.tile([16, 128], F32, name="e8_sb", tag="e8_sb")
            nc.sync.dma_start(out=e8_sb, in_=emats[0:16, :])
            e16_sb = constsS.tile([8, 128], F32, name="e16_sb", tag="e16_sb")
            nc.sync.dma_start(out=e16_sb, in_=emats[32:40, :])
            e4_sb = constsS.tile([32, 128], F32, name="e4_sb", tag="e4_sb")
            nc.sync.dma_start(out=e4_sb, in_=emats[64:96, :])
            hv_sb = constsS.tile([128, 2], F32, name="hv_sb", tag="hv_sb")
            nc.sync.dma_start(out=hv_sb, in_=bass.AP(tensor=hvd, offset=0, ap=[[0, 128], [1, 2]]))
            wt_sb = constsW.tile([128, 23, H2], BF16, name="wt_sb", tag="wt_sb")
            nc.sync.dma_start(out=wt_sb, in_=wtail[:, :, :])
            epsT = constsS.tile([32, 1], F32, name="epsT", tag="epsT")
            nc.vector.memset(epsT, EPS)
            ident = constsC.tile([128, 128], F32, name="ident", tag="ident")
            make_identity(nc, ident)

            S.update(vec=vec_sb, eps=epsT, e16=e16_sb, e4=e4_sb, hv=hv_sb,
                     g4=gm_sb[:, 24:56], wt=wt_sb)

            def vcol(r):
                return vec_sb[:, r:r + 1]
            g8_sb = gm_sb[:, 0:16]
            g16_sb = gm_sb[:, 16:24]

            # ---- conv1 + GN1 (fully local) + ReLU + transpose ----
            h_sb = [[bigres.tile([128, T], F32, name=f"h{b}{mt}", tag=f"h{b}{mt}") for mt in range(2)] for b in range(B)]
            hT_sb = [[bigres.tile([128, H1], BF16, name=f"ht{b}{tt}", tag=f"ht{b}{tt}") for tt in range(2)] for b in range(B)]
            st1 = ver.tile([16, B, 2, 2], F32, name="st1", tag="st1")
            rm1 = [[ver.tile([16, 2], F32, name=f"rm1_{b}{mt}", tag=f"rm1_{b}{mt}") for mt in range(2)] for b in range(B)]
            ph = {}
            for mt in range(2):
                ph[mt] = psA.tile([128, B, T], F32, name="mm", tag="mm")
                for idx in range(12):
                    j, ct = idx // 4, idx % 4
                    nc.tensor.matmul(
                        ph[mt],
                        c1w_sb[:, idx, mt * 128:(mt + 1) * 128],
                        x_sb[:, ct, :, j:j + T],
                        start=(idx == 0), stop=(idx == 11),
                    )
                for b in range(B):
                    stats_from(ph[mt][:, b, :], vcol(V_C1B + mt), g8_sb, 16, st1[:, b, mt, :])
            for b in range(B):
                for mt in range(2):
                    gn_finalize(st1[:, b, mt, :], 16, rm1[b][mt], epsT)
                    gn_apply(e8_sb, 16, rm1[b][mt], vcol(V_GN1G + mt), vcol(V_GN1B + mt),
                             vcol(V_C1B + mt), ph[mt][:, b, :], h_sb[b][mt], AF.Relu)
                for tt in range(2):
                    for mt in range(2):
                        pt = psA.tile([128, 128], F32, name="mm", tag="mm")
                        nc.tensor.transpose(pt, h_sb[b][mt][:, tt * 128:(tt + 1) * 128], ident)
                        nc.vector.tensor_copy(hT_sb[b][tt][:, mt * 128:(mt + 1) * 128], pt)

            # ---- sampling GEMM (fused into SBUF) ----
            samp_sb = [[bigres.tile([128, N, COLS], BF16, name=f"samp{b}{ct}", tag=f"samp{b}{ct}") for ct in range(2)] for b in range(B)]
            for ng in range(NGRP):
                mt_t = mstream.tile([128, NG, 2, COLS], BF16, name="mchunk", tag="mchunk")
                nc.sync.dma_start(out=mt_t, in_=maskin[ng])
                for b in range(B):
                    for ct in range(2):
                        ps = [psA.tile([128, COLS], F32, name="mm", tag="mm") for _ in range(NG)]
                        for tt in range(2):
                            for ni in range(NG):
                                nc.tensor.matmul(
                                    ps[ni],
                                    hT_sb[b][tt][:, ct * 128:(ct + 1) * 128],
                                    mt_t[:, ni, tt, :],
                                    start=(tt == 0), stop=(tt == 1),
                                )
                        for ni in range(NG):
                            if ni % 2 == 0:
                                nc.vector.tensor_copy(samp_sb[b][ct][:, ng * NG + ni, :], ps[ni])
                            else:
                                nc.scalar.activation(out=samp_sb[b][ct][:, ng * NG + ni, :],
                                                     in_=ps[ni], func=AF.Copy)

            # ---- Conv3d reduction (GEMM2, r3d streamed per-ot) + GN3 stats ----
            y_sb = [[ver.tile([128, COLS], BF16, name=f"y{b}{ot}", tag=f"y{b}{ot}") for ot in range(4)] for b in range(B)]
            st3 = ver.tile([8, B, 4, 2], F32, name="st3", tag="st3")
            st3g = ver.tile([8, B, 4, 2], F32, name="st3g", tag="st3g")
            for ot in range(4):
                r3c = r3ds.tile([128, 64, 128], BF16, name="r3c", tag="r3c")
                nc.scalar.dma_start(out=r3c, in_=r3dw[:, :, ot * 128:(ot + 1) * 128])
                pys = [psA.tile([128, COLS], F32, name="mm", tag="mm") for _ in range(B)]
                for kk in range(64):
                    n, ct = kk // 2, kk % 2
                    for b in range(B):
                        nc.tensor.matmul(
                            pys[b],
                            r3c[:, kk, :],
                            samp_sb[b][ct][:, n, :],
                            start=(kk == 0), stop=(kk == 63),
                        )
                for b in range(B):
                    nc.vector.tensor_copy(y_sb[b][ot], pys[b])
                    stats_from(y_sb[b][ot][:, OWN_LO:OWN_HI], vcol(V_R3DB + ot),
                               g16_sb, 8, st3[:, b, ot, :])
            ar3i = dram.tile([8, B, 4, 2], F32, name=f"ar3i{k}", tag=f"ar3i{k}")
            ar3o = dram.tile([8, B, 4, 2], F32, name=f"ar3o{k}", tag=f"ar3o{k}", addr_space="Shared")
            nc.gpsimd.dma_start(out=ar3i[:, :, :, :], in_=st3[:, :, :, :])
            if no_cc:
                nc.gpsimd.dma_start(out=ar3o[:, :, :, :], in_=ar3i[:, :, :, :])
            else:
                nc.gpsimd.collective_compute("AllReduce", ALU.add, replica_groups=RG,
                                             ins=[ar3i.opt()], outs=[ar3o.opt()])
            nc.gpsimd.dma_start(out=st3g[:, :, :, :], in_=ar3o[:, :, :, :])
            S.update(y=y_sb, st3g=st3g)
            return S

        def stage_T1(S):
            k, vec_sb, epsT = S["k"], S["vec"], S["eps"]
            y_sb = S["y"]
            gn_batch(S["e16"], 8, S["st3g"].rearrange("g b o s -> g (b o) s"), 8,
                     V_BG3G, V_BG3B, V_BG3C,
                     [y_sb[b][ot] for b in range(B) for ot in range(4)],
                     [y_sb[b][ot] for b in range(B) for ot in range(4)], AF.Relu,
                     vec_sb, epsT)
            # ---- 1x1 reduction conv (r2d) + GN2 stats ----
            r2d_sb = S["wt"][:, WT_R2D:WT_R2D + 4, :]
            pfx = [ver.tile([128, COLS], F32, name=f"pfx{b}", tag=f"pfx{b}") for b in range(B)]
            st2 = ver.tile([32, B, 2], F32, name="st2", tag="st2")
            st2g = ver.tile([32, B, 2], F32, name="st2g", tag="st2g")
            for b in range(B):
                pf = psA.tile([128, COLS], F32, name="mm", tag="mm")
                for ot in range(4):
                    nc.tensor.matmul(pf, r2d_sb[:, ot, :], y_sb[b][ot],
                                     start=(ot == 0), stop=(ot == 3))
                stats_from(pf[:, OWN_LO:OWN_HI], vec_sb[:, V_R2DB:V_R2DB + 1], S["g4"], 32, st2[:, b, :])
                nc.vector.tensor_copy(pfx[b], pf)
            ar2i = dram.tile([32, B, 2], F32, name=f"ar2i{k}", tag=f"ar2i{k}")
            ar2o = dram.tile([32, B, 2], F32, name=f"ar2o{k}", tag=f"ar2o{k}", addr_space="Shared")
            nc.gpsimd.dma_start(out=ar2i[:, :, :], in_=st2[:, :, :])
            if no_cc:
                nc.gpsimd.dma_start(out=ar2o[:, :, :], in_=ar2i[:, :, :])
            else:
                nc.gpsimd.collective_compute("AllReduce", ALU.add, replica_groups=RG,
                                             ins=[ar2i.opt()], outs=[ar2o.opt()])
            nc.gpsimd.dma_start(out=st2g[:, :, :], in_=ar2o[:, :, :])
            S.update(pfx=pfx, st2g=st2g)

        def stage_T2(S):
            k, vec_sb, epsT = S["k"], S["vec"], S["eps"]
            # ---- GN2 apply -> f, halo scale, heads 3x3 conv + stats ----
            f_sb = [ver.tile([128, TH, W + 2], BF16, name=f"f{b}", tag=f"f{b}") for b in range(B)]
            for b in range(B):
                nc.vector.memset(f_sb[b], 0.0)
            gn_batch(S["e4"], 32, S["st2g"], 2, V_BG2G, V_BG2B, V_BG2C,
                     [S["pfx"][b] for b in range(B)],
                     [f_sb[b][:, :, 1:W + 1] for b in range(B)], AF.Relu,
                     vec_sb, epsT)
            hv_sb = S["hv"]
            for b in range(B):
                nc.vector.tensor_scalar_mul(f_sb[b][:, 0, 1:W + 1], f_sb[b][:, 0, 1:W + 1], hv_sb[:, 0:1])
                nc.vector.tensor_scalar_mul(f_sb[b][:, TH - 1, 1:W + 1], f_sb[b][:, TH - 1, 1:W + 1], hv_sb[:, 1:2])

            s1w_sb = S["wt"][:, WT_S1:WT_S1 + 9, :]
            e1w_sb = S["wt"][:, WT_E1:WT_E1 + 9, :]
            sth = ver.tile([32, B, 2, 2], F32, name="sth", tag="sth")
            sthg = ver.tile([32, B, 2, 2], F32, name="sthg", tag="sthg")
            phx = [[ver.tile([128, TOWN * W], F32, name=f"phx{b}{hd}", tag=f"phx{b}{hd}") for hd in range(2)] for b in range(B)]
            for hd in range(2):
                w_sb = s1w_sb if hd == 0 else e1w_sb
                phd = {}
                for b in range(B):
                    phd[b] = psA.tile([128, TOWN * W], F32, name="mm", tag="mm")
                for tap in range(9):
                    kt, kw = tap // 3, tap % 3
                    for b in range(B):
                        nc.tensor.matmul(phd[b], w_sb[:, tap, :],
                                         f_sb[b][:, kt:kt + TOWN, kw:kw + W],
                                         start=(tap == 0), stop=(tap == 8))
                for b in range(B):
                    stats_from(phd[b], vec_sb[:, V_S1B + hd:V_S1B + hd + 1], S["g4"], 32, sth[:, b, hd, :])
                    nc.vector.tensor_copy(phx[b][hd], phd[b])
            arhi = dram.tile([32, B, 2, 2], F32, name=f"arhi{k}", tag=f"arhi{k}")
            arho = dram.tile([32, B, 2, 2], F32, name=f"arho{k}", tag=f"arho{k}", addr_space="Shared")
            nc.gpsimd.dma_start(out=arhi[:, :, :, :], in_=sth[:, :, :, :])
            if no_cc:
                nc.gpsimd.dma_start(out=arho[:, :, :, :], in_=arhi[:, :, :, :])
            else:
                nc.gpsimd.collective_compute("AllReduce", ALU.add, replica_groups=RG,
                                             ins=[arhi.opt()], outs=[arho.opt()])
            nc.gpsimd.dma_start(out=sthg[:, :, :, :], in_=arho[:, :, :, :])
            S.update(phx=phx, sthg=sthg)

        def stage_T3(S):
            vec_sb, epsT = S["vec"], S["eps"]
            hact = [[ver.tile([128, TOWN * W], BF16, name=f"hact{b}{hd}", tag=f"hact{b}{hd}") for hd in range(2)] for b in range(B)]
            o_t = [[ver.tile([1, TOWN * W], F32, name=f"o{b}{hd}", tag=f"o{b}{hd}") for hd in range(2)] for b in range(B)]
            gn_batch(S["e4"], 32, S["sthg"].rearrange("g b h s -> g (b h) s"), 4,
                     V_BHG, V_BHB, V_BHC,
                     [S["phx"][b][hd] for b in range(B) for hd in range(2)],
                     [hact[b][hd] for b in range(B) for hd in range(2)], AF.Relu,
                     vec_sb, epsT)
            s2w_sb = S["wt"][:, WT_S2, 0:2]
            for b in range(B):
                for hd in range(2):
                    po = psS.tile([1, TOWN * W], F32, name="pst", tag="pst")
                    nc.tensor.matmul(po, s2w_sb[:, hd:hd + 1], hact[b][hd], start=True, stop=True)
                    nc.scalar.activation(out=o_t[b][hd], in_=po, func=AF.Sigmoid,
                                         bias=vec_sb[0:1, V_S2B + hd:V_S2B + hd + 1], scale=1.0)
                    nc.gpsimd.dma_start(out=outd[b, hd], in_=o_t[b][hd])

        states = {}
        for k in range(iters):
            states[k] = stage_F(k)
            if k >= 1:
                stage_T1(states[k - 1])
            if k >= 2:
                stage_T2(states[k - 2])
            if k >= 3:
                stage_T3(states.pop(k - 3))
        # drain
        if iters >= 1:
            stage_T1(states[iters - 1])
        if iters >= 2:
            stage_T2(states[iters - 2])
        if iters >= 3:
            stage_T3(states.pop(iters - 3))
        stage_T2(states[iters - 1])
        if iters >= 2:
            stage_T3(states.pop(iters - 2))
        stage_T3(states.pop(iters - 1))

    nc.compile()
    return nc


_module_cache = {}


def _get_module():
    if "nc" not in _module_cache:
        _module_cache["nc"] = _build()
    return _module_cache["nc"]


def _prep(inputs):
    def f32(a):
        return np.ascontiguousarray(np.asarray(a, dtype=np.float32))

    x = f32(inputs["x"])
    mask = f32(inputs["sample_mask"]).reshape(T, N, T, W)

    c1_w = f32(inputs["c1_w"])
    r3d_w = f32(inputs["r3d_w"])[:, :, :, 0, 0]
    r2d_w = f32(inputs["r2d_w"])[:, :, 0, 0]
    s1_w = f32(inputs["s1_w"])
    e1_w = f32(inputs["e1_w"])
    s2_w = f32(inputs["s2_w"])[0, :, 0, 0]
    e2_w = f32(inputs["e2_w"])[0, :, 0, 0]

    x_h = x.astype(BFNP)

    # conv1 weights: [c, j*4+ct, m] = c1_w[m, ct*128+c, j]
    a = c1_w.transpose(1, 2, 0).reshape(4, 128, 3, H1)
    c1w_h = a.transpose(1, 2, 0, 3).reshape(128, 12, H1).astype(BFNP)

    # r3d weights: [c, n*2+ct, o] = r3d_w[o, ct*128+c, n]
    a = r3d_w.transpose(1, 2, 0).reshape(2, 128, N, H3)
    r3d_h = np.ascontiguousarray(a.transpose(1, 2, 0, 3).reshape(128, 64, H3)).astype(BFNP)

    # packed tail weights [128, 23, 128]
    wtail = np.zeros((128, 23, H2), np.float32)
    wtail[:, WT_R2D:WT_R2D + 4, :] = r2d_w.T.reshape(4, 128, H2).transpose(1, 0, 2)
    wtail[:, WT_S1:WT_S1 + 9, :] = s1_w.transpose(1, 2, 3, 0).reshape(128, 9, H2)
    wtail[:, WT_E1:WT_E1 + 9, :] = e1_w.transpose(1, 2, 3, 0).reshape(128, 9, H2)
    wtail[:, WT_S2, 0] = s2_w
    wtail[:, WT_S2, 1] = e2_w
    wtail_h = wtail.astype(BFNP)

    ch = np.arange(128)
    g8 = (ch[:, None] // 8 == np.arange(16)[None, :]).astype(np.float32)
    g16 = (ch[:, None] // 16 == np.arange(8)[None, :]).astype(np.float32)
    g4 = (ch[:, None] // 4 == np.arange(32)[None, :]).astype(np.float32)
    gmats = np.concatenate([g8 / 8.0, g16 / (16.0 * 8), g4 / (4.0 * 8)], axis=1)
    emats = np.zeros((96, 128), np.float32)
    emats[0:16] = g8.T
    emats[32:40] = g16.T
    emats[64:96] = g4.T

    vecs = np.zeros((NVEC, 128), np.float32)
    vecs[V_C1B:V_C1B + 2] = f32(inputs["c1_b"]).reshape(2, 128)
    vecs[V_GN1G:V_GN1G + 2] = f32(inputs["gn1_g"]).reshape(2, 128)
    vecs[V_GN1B:V_GN1B + 2] = f32(inputs["gn1_b"]).reshape(2, 128)
    vecs[V_R3DB:V_R3DB + 4] = f32(inputs["r3d_b"]).reshape(4, 128)
    vecs[V_GN3G:V_GN3G + 4] = f32(inputs["gn3_g"]).reshape(4, 128)
    vecs[V_GN3B:V_GN3B + 4] = f32(inputs["gn3_b"]).reshape(4, 128)
    vecs[V_R2DB] = f32(inputs["r2d_b"])
    vecs[V_GN2G] = f32(inputs["gn2_g"])
    vecs[V_GN2B] = f32(inputs["gn2_b"])
    vecs[V_S1B] = f32(inputs["s1_b"])
    vecs[V_E1B] = f32(inputs["e1_b"])
    vecs[V_SGNG] = f32(inputs["sgn_g"])
    vecs[V_SGNB] = f32(inputs["sgn_b"])
    vecs[V_EGNG] = f32(inputs["egn_g"])
    vecs[V_EGNB] = f32(inputs["egn_b"])
    vecs[V_S2B] = f32(inputs["s2_b"])[0]
    vecs[V_E2B] = f32(inputs["e2_b"])[0]
    gn1g2 = f32(inputs["gn1_g"]).reshape(2, 128)
    gn1b2 = f32(inputs["gn1_b"]).reshape(2, 128)
    c1b2 = f32(inputs["c1_b"]).reshape(2, 128)
    for i, (b, mt) in enumerate([(b, mt) for b in range(B) for mt in range(2)]):
        vecs[V_BG1G + i] = gn1g2[mt]
        vecs[V_BG1B + i] = gn1b2[mt]
        vecs[V_BG1C + i] = c1b2[mt]
    gn3g4 = f32(inputs["gn3_g"]).reshape(4, 128)
    gn3b4 = f32(inputs["gn3_b"]).reshape(4, 128)
    r3db4 = f32(inputs["r3d_b"]).reshape(4, 128)
    for i, (b, ot) in enumerate([(b, ot) for b in range(B) for ot in range(4)]):
        vecs[V_BG3G + i] = gn3g4[ot]
        vecs[V_BG3B + i] = gn3b4[ot]
        vecs[V_BG3C + i] = r3db4[ot]
    for b in range(B):
        vecs[V_BG2G + b] = f32(inputs["gn2_g"])
        vecs[V_BG2B + b] = f32(inputs["gn2_b"])
        vecs[V_BG2C + b] = f32(inputs["r2d_b"])
    hg = [f32(inputs["sgn_g"]), f32(inputs["egn_g"])]
    hb = [f32(inputs["sgn_b"]), f32(inputs["egn_b"])]
    hc = [f32(inputs["s1_b"]), f32(inputs["e1_b"])]
    for i, (b, hd) in enumerate([(b, hd) for b in range(B) for hd in range(2)]):
        vecs[V_BHG + i] = hg[hd]
        vecs[V_BHB + i] = hb[hd]
        vecs[V_BHC + i] = hc[hd]

    shared = {
        "x_in": x_h, "c1w": c1w_h, "r3dw": r3d_h, "wtail": wtail_h,
        "gmats": gmats, "emats": emats, "vecs": vecs,
    }

    in_maps = []
    for k in range(NCORES):
        t0 = k * TOWN
        tlo = t0 - 1
        m4 = np.zeros((T, N, TH, W), np.float32)
        slo, shi = max(0, tlo), min(T, t0 + TOWN + 1)
        m4[:, :, slo - tlo: shi - tlo, :] = mask[:, :, slo:shi, :]
        # -> [group, partition, ni, tau_tile, col]
        m_h = np.ascontiguousarray(
            m4.reshape(T, N, COLS).transpose(1, 0, 2)       # (N, T, COLS)
              .reshape(NGRP, NG, 2, 128, COLS)
              .transpose(0, 3, 1, 2, 4)                      # (NGRP, 128, NG, 2, COLS)
        ).astype(BFNP)
        hv = np.array([1.0 if k > 0 else 0.0, 1.0 if k < NCORES - 1 else 0.0], np.float32)
        in_maps.append(dict(shared, mask_in=m_h, hv=hv))
    return in_maps


def kernel(**inputs) -> np.ndarray:
    nc = _get_module()
    in_maps = _prep(inputs)
    from concourse.bass_utils import run_bass_kernel_spmd
    res = run_bass_kernel_spmd(nc, in_maps, list(range(NCORES)))
    full = np.zeros((B, 2, T, W), np.float32)
    for k in range(NCORES):
        full[:, :, k * TOWN:(k + 1) * TOWN, :] = res.results[k]["out"]
    return full



# revision 14
# speedup vs baseline: 1.1053x; 1.1053x over previous
"""BEM (boundary evaluation module) Trainium2 kernel.

Strategy: shard the T=256 axis across 8 NeuronCores (32 own columns plus one
recomputed halo column on each side).  Each core runs the full pipeline on its
t-slice; the sampling GEMM and the Conv3d reduction are fused in SBUF so the
(B,C,N,T,W) intermediate never touches HBM.  GroupNorm statistics that span
the sharded axis are combined with three tiny HBM AllReduces.
"""

import os
import sys

import numpy as np

for _p in ("/opt/trn_rl_repo", "/root/.axon_site/_ro/trn_rl_repo"):
    if _p not in sys.path:
        sys.path.append(_p)

import ml_dtypes  # noqa: E402
import concourse.bass as bass  # noqa: E402
import concourse.bacc as bacc  # noqa: E402
import concourse.tile as tile  # noqa: E402
import concourse.mybir as mybir  # noqa: E402
from contextlib import ExitStack  # noqa: E402
from concourse.masks import make_identity  # noqa: E402

F32 = mybir.dt.float32
BF16 = mybir.dt.bfloat16
AF = mybir.ActivationFunctionType
ALU = mybir.AluOpType
BFNP = ml_dtypes.bfloat16

B = 2
DIM = 512
T = 256
H1 = 256
H3 = 512
H2 = 128
N = 32
W = 8
NCORES = 8
TOWN = T // NCORES          # 32 own t columns per core
TH = TOWN + 2               # with halo
COLS = TH * W               # 272
OWN_LO, OWN_HI = W, W + TOWN * W  # own column range inside the 272
EPS = 1e-5
NG = int(os.environ.get("KBEM_NG", "4"))  # mask n's per streamed group
NGRP = N // NG

# rows of the packed per-channel vector table
V_C1B = 0          # 2 rows (mt)
V_GN1G = 2         # 2
V_GN1B = 4         # 2
V_R3DB = 6         # 4 (ot)
V_GN3G = 10        # 4
V_GN3B = 14        # 4
V_R2DB = 18
V_GN2G = 19
V_GN2B = 20
V_S1B = 21
V_E1B = 22
V_SGNG = 23
V_SGNB = 24
V_EGNG = 25
V_EGNB = 26
V_S2B = 27         # s2 bias broadcast
V_E2B = 28
# batched per-instance tables (columns in instance order)
V_BG1G = 29        # 4: i = b*2+mt -> gn1_g[mt]
V_BG1B = 33
V_BG1C = 37        # c1_b[mt]
V_BG3G = 41        # 8: i = b*4+ot -> gn3_g[ot]
V_BG3B = 49
V_BG3C = 57        # r3d_b[ot]
V_BG2G = 65        # 2: i = b -> gn2_g
V_BG2B = 67
V_BG2C = 69        # r2d_b
V_BHG = 71         # 4: i = b*2+hd -> sgn_g/egn_g
V_BHB = 75
V_BHC = 79         # s1_b/e1_b
NVEC = 83

# wtail packing (bf16, [128, 23, 128]): r2d 0:4, s1 4:13, e1 13:22, s2 22
WT_R2D = 0
WT_S1 = 4
WT_E1 = 13
WT_S2 = 22

RG = [list(range(NCORES))]


def _build(iters=None):
    """Depth-4 software-pipelined emission: slot k emits F(k), T1(k-1),
    T2(k-2), T3(k-3) so the tensor queue never head-of-line blocks on the
    three GroupNorm AllReduce chains.

      F : loads, conv1+GN1(local)+transpose, sampling GEMM, Conv3d
          reduction, GN3 stats -> AllReduce#1
      T1: GN3 apply, r2d 1x1, GN2 stats -> AllReduce#2
      T2: GN2 apply -> f, halo scale, head 3x3 convs, head GN stats
          -> AllReduce#3
      T3: head GN apply, 1x1 sigmoid heads, output DMA
    """
    no_cc = bool(int(os.environ.get("KBEM_NO_CC", "0")))
    if iters is None:
        iters = int(os.environ.get("KBEM_ITERS", "1"))
    nc = bacc.Bacc("TRN2", target_bir_lowering=False, debug=False)

    xin = nc.declare_dram_parameter("x_in", [B, DIM, T], BF16, isOutput=False)
    maskin = nc.declare_dram_parameter("mask_in", [NGRP, 128, NG, 2, COLS], BF16, isOutput=False)
    c1w = nc.declare_dram_parameter("c1w", [128, 12, H1], BF16, isOutput=False)
    r3dw = nc.declare_dram_parameter("r3dw", [128, 64, H3], BF16, isOutput=False)
    wtail = nc.declare_dram_parameter("wtail", [128, 23, H2], BF16, isOutput=False)
    gmats = nc.declare_dram_parameter("gmats", [128, 56], F32, isOutput=False)
    emats = nc.declare_dram_parameter("emats", [96, 128], F32, isOutput=False)
    vecsd = nc.declare_dram_parameter("vecs", [NVEC, 128], F32, isOutput=False)
    hvd = nc.declare_dram_parameter("hv", [2], F32, isOutput=False)
    outd = nc.declare_dram_parameter("out", [B, 2, TOWN, W], F32, isOutput=True)

    with tile.TileContext(nc) as tc, ExitStack() as ctx:
        dram = ctx.enter_context(tc.tile_pool(name="dram", bufs=1, space="DRAM"))
        # version-k consts live until T3(k) at slot k+3 -> 4 rotating buffers
        constsS = ctx.enter_context(tc.tile_pool(name="constsS", bufs=4))
        constsW = ctx.enter_context(tc.tile_pool(name="constsW", bufs=4))
        # F-internal consts only need the F(k) / F(k+1) overlap
        constsC = ctx.enter_context(tc.tile_pool(name="constsC", bufs=2))
        bigres = ctx.enter_context(tc.tile_pool(name="bigres", bufs=1))
        r3ds = ctx.enter_context(tc.tile_pool(name="r3ds", bufs=int(os.environ.get("KBEM_R3DBUFS", "2"))))
        ver = ctx.enter_context(tc.tile_pool(name="ver", bufs=2))
        mstream = ctx.enter_context(tc.tile_pool(name="mstream", bufs=int(os.environ.get("KBEM_MBUFS", "2"))))
        small = ctx.enter_context(tc.tile_pool(name="small", bufs=16))
        psA = ctx.enter_context(tc.tile_pool(name="psA", bufs=int(os.environ.get("KBEM_PSA", "6")), space="PSUM"))
        psS = ctx.enter_context(tc.tile_pool(name="psS", bufs=int(os.environ.get("KBEM_PSS", "2")), space="PSUM"))

        # ---- GroupNorm helpers (version-bound via explicit args) ----
        def stats_from(src_ap, bias_ap, G, gdim, dst):
            """Scaled-group [mean, E[x^2]] of (src+bias) -> dst (gdim,2).
            G is pre-scaled by 1/(group_partitions * participating_cores) so
            the matmul (plus the later AllReduce) averages directly."""
            st6 = small.tile([128, 6], F32, name="st6", tag="st6")
            nc.vector.bn_stats(out=st6, in_=src_ap)
            mv = small.tile([128, 2], F32, name="mv", tag="mv")
            nc.vector.bn_aggr(out=mv, in_=st6)
            s12 = small.tile([128, 2], F32, name="s12", tag="s12")
            nc.vector.tensor_scalar_add(s12[:, 0:1], mv[:, 0:1], bias_ap)
            sq = small.tile([128, 1], F32, name="sq", tag="sq")
            nc.vector.tensor_mul(sq, s12[:, 0:1], s12[:, 0:1])
            nc.vector.tensor_add(s12[:, 1:2], mv[:, 1:2], sq)
            pg = psS.tile([gdim, 2], F32, name="pst", tag="pst")
            nc.tensor.matmul(pg, G[:, :], s12, start=True, stop=True)
            nc.vector.tensor_copy(dst, pg)

        def gn_finalize(stats_slice, gdim, rm_dst, epsT):
            var = small.tile([32, 1], F32, name="var", tag="var")[:gdim]
            sq = small.tile([32, 1], F32, name="sqg", tag="sqg")[:gdim]
            nc.vector.tensor_mul(sq, stats_slice[:, 0:1], stats_slice[:, 0:1])
            nc.vector.tensor_sub(var, stats_slice[:, 1:2], sq)
            nc.scalar.activation(out=var, in_=var, func=AF.Sqrt, bias=epsT[:gdim], scale=1.0)
            nc.vector.reciprocal(rm_dst[:, 0:1], var)
            nc.vector.tensor_copy(rm_dst[:, 1:2], stats_slice[:, 0:1])

        def gn_apply(E, gdim, rm_slice, gamma_ap, beta_ap, cbias_ap, src_ap, out_ap, func):
            pb = psS.tile([128, 2], F32, name="pst", tag="pst")
            nc.tensor.matmul(pb, E[:, :], rm_slice, start=True, stop=True)
            scale = small.tile([128, 1], F32, name="scale", tag="scale")
            nc.vector.tensor_mul(scale, pb[:, 0:1], gamma_ap)
            t1 = small.tile([128, 1], F32, name="t1", tag="t1")
            nc.vector.tensor_sub(t1, cbias_ap, pb[:, 1:2])
            t2 = small.tile([128, 1], F32, name="t2", tag="t2")
            nc.vector.tensor_mul(t2, t1, scale)
            bias = small.tile([128, 1], F32, name="bias", tag="bias")
            nc.vector.tensor_add(bias, t2, beta_ap)
            nc.scalar.activation(out=out_ap, in_=src_ap, func=func, bias=bias, scale=scale)

        def gn_batch(E, gdim, stg_view, ni, gG, gB, gC, srcs, outs, func, vec_sb, epsT):
            rm = small.tile([32, 8, 2], F32, name="rmb", tag="rmb")[:gdim, :ni, :]
            sq = small.tile([32, 8], F32, name="sqb", tag="sqb")[:gdim, :ni]
            var = small.tile([32, 8], F32, name="varb", tag="varb")[:gdim, :ni]
            nc.vector.tensor_mul(sq, stg_view[:, :, 0], stg_view[:, :, 0])
            nc.vector.tensor_sub(var, stg_view[:, :, 1], sq)
            nc.scalar.activation(out=var, in_=var, func=AF.Sqrt, bias=epsT[:gdim], scale=1.0)
            nc.vector.reciprocal(rm[:, :, 0], var)
            nc.vector.tensor_copy(rm[:, :, 1], stg_view[:, :, 0])
            pb = psS.tile([128, 8, 2], F32, name="pst", tag="pst")[:, :ni, :]
            nc.tensor.matmul(pb, E[:, :], rm, start=True, stop=True)
            scale = small.tile([128, 8], F32, name="scaleb", tag="scaleb")[:, :ni]
            bias = small.tile([128, 8], F32, name="biasb", tag="biasb")[:, :ni]
            t1 = small.tile([32, 8], F32, name="t1c", tag="t1c")[:, :ni] if False else small.tile([128, 8], F32, name="t1b", tag="t1b")[:, :ni]
            nc.vector.tensor_mul(scale, pb[:, :, 0], vec_sb[:, gG:gG + ni])
            nc.vector.tensor_sub(t1, vec_sb[:, gC:gC + ni], pb[:, :, 1])
            nc.vector.tensor_mul(t1, t1, scale)
            nc.vector.tensor_add(bias, t1, vec_sb[:, gB:gB + ni])
            for i in range(ni):
                nc.scalar.activation(out=outs[i], in_=srcs[i], func=func,
                                     bias=bias[:, i:i + 1], scale=scale[:, i:i + 1])

        def stage_F(k):
            S = {"k": k}
            # ---- loads ----
            x_sb = bigres.tile([128, 4, B, T + 2], BF16, name="x_sb", tag="x_sb")
            nc.vector.memset(x_sb[:, :, :, 0:1], 0.0)
            nc.vector.memset(x_sb[:, :, :, T + 1:T + 2], 0.0)
            for b in range(B):
                nc.sync.dma_start(
                    out=x_sb[:, :, b, 1:T + 1],
                    in_=bass.AP(tensor=xin, offset=b * DIM * T,
                                ap=[[T, 128], [128 * T, 4], [1, T]]))
            c1w_sb = constsC.tile([128, 12, H1], BF16, name="c1w_sb", tag="c1w_sb")
            nc.sync.dma_start(out=c1w_sb, in_=c1w[:, :, :])
            vec_sb = constsS.tile([128, NVEC], F32, name="vec_sb", tag="vec_sb")
            nc.sync.dma_start(out=vec_sb, in_=bass.AP(tensor=vecsd, offset=0, ap=[[1, 128], [128, NVEC]]))
            gm_sb = constsS.tile([128, 56], F32, name="gm_sb", tag="gm_sb")
            nc.sync.dma_start(out=gm_sb, in_=gmats[:, :])
            e8_sb = constsC.tile([16, 128], F32, name="e8_sb", tag="e8_sb")
            nc.sync.dma_start(out=e8_sb, in_=emats[0:16, :])
            e16_sb = constsS.tile([8, 128], F32, name="e16_sb", tag="e16_sb")
            nc.sync.dma_start(out=e16_sb, in_=emats[32:40, :])
            e4_sb = constsS.tile([32, 128], F32, name="e4_sb", tag="e4_sb")
            nc.sync.dma_start(out=e4_sb, in_=emats[64:96, :])
            hv_sb = constsS.tile([128, 2], F32, name="hv_sb", tag="hv_sb")
            nc.sync.dma_start(out=hv_sb, in_=bass.AP(tensor=hvd, offset=0, ap=[[0, 128], [1, 2]]))
            wt_sb = constsW.tile([128, 23, H2], BF16, name="wt_sb", tag="wt_sb")
            nc.sync.dma_start(out=wt_sb, in_=wtail[:, :, :])
            epsT = constsS.tile([32, 1], F32, name="epsT", tag="epsT")
            nc.vector.memset(epsT, EPS)
            ident = constsC.tile([128, 128], F32, name="ident", tag="ident")
            make_identity(nc, ident)

            S.update(vec=vec_sb, eps=epsT, e16=e16_sb, e4=e4_sb, hv=hv_sb,
                     g4=gm_sb[:, 24:56], wt=wt_sb)

            def vcol(r):
                return vec_sb[:, r:r + 1]
            g8_sb = gm_sb[:, 0:16]
            g16_sb = gm_sb[:, 16:24]

            # ---- conv1 + GN1 (fully local) + ReLU + transpose ----
            h_sb = [[bigres.tile([128, T], F32, name=f"h{b}{mt}", tag=f"h{b}{mt}") for mt in range(2)] for b in range(B)]
            hT_sb = [[bigres.tile([128, H1], BF16, name=f"ht{b}{tt}", tag=f"ht{b}{tt}") for tt in range(2)] for b in range(B)]
            st1 = ver.tile([16, B, 2, 2], F32, name="st1", tag="st1")
            rm1 = [[ver.tile([16, 2], F32, name=f"rm1_{b}{mt}", tag=f"rm1_{b}{mt}") for mt in range(2)] for b in range(B)]
            ph = {}
            for mt in range(2):
                ph[mt] = psA.tile([128, B, T], F32, name="mm", tag="mm")
                for idx in range(12):
                    j, ct = idx // 4, idx % 4
                    nc.tensor.matmul(
                        ph[mt],
                        c1w_sb[:, idx, mt * 128:(mt + 1) * 128],
                        x_sb[:, ct, :, j:j + T],
                        start=(idx == 0), stop=(idx == 11),
                    )
                for b in range(B):
                    stats_from(ph[mt][:, b, :], vcol(V_C1B + mt), g8_sb, 16, st1[:, b, mt, :])
            for b in range(B):
                for mt in range(2):
                    gn_finalize(st1[:, b, mt, :], 16, rm1[b][mt], epsT)
                    gn_apply(e8_sb, 16, rm1[b][mt], vcol(V_GN1G + mt), vcol(V_GN1B + mt),
                             vcol(V_C1B + mt), ph[mt][:, b, :], h_sb[b][mt], AF.Relu)
                for tt in range(2):
                    for mt in range(2):
                        pt = psA.tile([128, 128], F32, name="mm", tag="mm")
                        nc.tensor.transpose(pt, h_sb[b][mt][:, tt * 128:(tt + 1) * 128], ident)
                        nc.vector.tensor_copy(hT_sb[b][tt][:, mt * 128:(mt + 1) * 128], pt)

            # ---- sampling GEMM (fused into SBUF) ----
            samp_sb = [[bigres.tile([128, N, COLS], BF16, name=f"samp{b}{ct}", tag=f"samp{b}{ct}") for ct in range(2)] for b in range(B)]
            for ng in range(NGRP):
                mt_t = mstream.tile([128, NG, 2, COLS], BF16, name="mchunk", tag="mchunk")
                nc.sync.dma_start(out=mt_t, in_=maskin[ng])
                for b in range(B):
                    for ct in range(2):
                        ps = [psA.tile([128, COLS], F32, name="mm", tag="mm") for _ in range(NG)]
                        for tt in range(2):
                            for ni in range(NG):
                                nc.tensor.matmul(
                                    ps[ni],
                                    hT_sb[b][tt][:, ct * 128:(ct + 1) * 128],
                                    mt_t[:, ni, tt, :],
                                    start=(tt == 0), stop=(tt == 1),
                                )
                        for ni in range(NG):
                            if ni % 2 == 0:
                                nc.vector.tensor_copy(samp_sb[b][ct][:, ng * NG + ni, :], ps[ni])
                            else:
                                nc.scalar.activation(out=samp_sb[b][ct][:, ng * NG + ni, :],
                                                     in_=ps[ni], func=AF.Copy)

            # ---- Conv3d reduction (GEMM2, r3d streamed per-ot) + GN3 stats ----
            y_sb = [[ver.tile([128, COLS], BF16, name=f"y{b}{ot}", tag=f"y{b}{ot}") for ot in range(4)] for b in range(B)]
            st3 = ver.tile([8, B, 4, 2], F32, name="st3", tag="st3")
            st3g = ver.tile([8, B, 4, 2], F32, name="st3g", tag="st3g")
            for ot in range(4):
                r3c = r3ds.tile([128, 64, 128], BF16, name="r3c", tag="r3c")
                nc.scalar.dma_start(out=r3c, in_=r3dw[:, :, ot * 128:(ot + 1) * 128])
                pys = [psA.tile([128, COLS], F32, name="mm", tag="mm") for _ in range(B)]
                for kk in range(64):
                    n, ct = kk // 2, kk % 2
                    for b in range(B):
                        nc.tensor.matmul(
                            pys[b],
                            r3c[:, kk, :],
                            samp_sb[b][ct][:, n, :],
                            start=(kk == 0), stop=(kk == 63),
                        )
                for b in range(B):
                    nc.vector.tensor_copy(y_sb[b][ot], pys[b])
                    stats_from(y_sb[b][ot][:, OWN_LO:OWN_HI], vcol(V_R3DB + ot),
                               g16_sb, 8, st3[:, b, ot, :])
            ar3i = dram.tile([8, B, 4, 2], F32, name=f"ar3i{k}", tag=f"ar3i{k}")
            ar3o = dram.tile([8, B, 4, 2], F32, name=f"ar3o{k}", tag=f"ar3o{k}", addr_space="Shared")
            nc.gpsimd.dma_start(out=ar3i[:, :, :, :], in_=st3[:, :, :, :])
            if no_cc:
                nc.gpsimd.dma_start(out=ar3o[:, :, :, :], in_=ar3i[:, :, :, :])
            else:
                nc.gpsimd.collective_compute("AllReduce", ALU.add, replica_groups=RG,
                                             ins=[ar3i.opt()], outs=[ar3o.opt()])
            nc.gpsimd.dma_start(out=st3g[:, :, :, :], in_=ar3o[:, :, :, :])
            S.update(y=y_sb, st3g=st3g)
            return S

        def stage_T1(S):
            k, vec_sb, epsT = S["k"], S["vec"], S["eps"]
            y_sb = S["y"]
            gn_batch(S["e16"], 8, S["st3g"].rearrange("g b o s -> g (b o) s"), 8,
                     V_BG3G, V_BG3B, V_BG3C,
                     [y_sb[b][ot] for b in range(B) for ot in range(4)],
                     [y_sb[b][ot] for b in range(B) for ot in range(4)], AF.Relu,
                     vec_sb, epsT)
            # ---- 1x1 reduction conv (r2d) + GN2 stats ----
            r2d_sb = S["wt"][:, WT_R2D:WT_R2D + 4, :]
            pfx = [ver.tile([128, COLS], F32, name=f"pfx{b}", tag=f"pfx{b}") for b in range(B)]
            st2 = ver.tile([32, B, 2], F32, name="st2", tag="st2")
            st2g = ver.tile([32, B, 2], F32, name="st2g", tag="st2g")
            for b in range(B):
                pf = psA.tile([128, COLS], F32, name="mm", tag="mm")
                for ot in range(4):
                    nc.tensor.matmul(pf, r2d_sb[:, ot, :], y_sb[b][ot],
                                     start=(ot == 0), stop=(ot == 3))
                stats_from(pf[:, OWN_LO:OWN_HI], vec_sb[:, V_R2DB:V_R2DB + 1], S["g4"], 32, st2[:, b, :])
                nc.vector.tensor_copy(pfx[b], pf)
            ar2i = dram.tile([32, B, 2], F32, name=f"ar2i{k}", tag=f"ar2i{k}")
            ar2o = dram.tile([32, B, 2], F32, name=f"ar2o{k}", tag=f"ar2o{k}", addr_space="Shared")
            nc.gpsimd.dma_start(out=ar2i[:, :, :], in_=st2[:, :, :])
            if no_cc:
                nc.gpsimd.dma_start(out=ar2o[:, :, :], in_=ar2i[:, :, :])
            else:
                nc.gpsimd.collective_compute("AllReduce", ALU.add, replica_groups=RG,
                                             ins=[ar2i.opt()], outs=[ar2o.opt()])
            nc.gpsimd.dma_start(out=st2g[:, :, :], in_=ar2o[:, :, :])
            S.update(pfx=pfx, st2g=st2g)

        def stage_T2(S):
            k, vec_sb, epsT = S["k"], S["vec"], S["eps"]
            # ---- GN2 apply -> f, halo scale, heads 3x3 conv + stats ----
            f_sb = [ver.tile([128, TH, W + 2], BF16, name=f"f{b}", tag=f"f{b}") for b in range(B)]
            for b in range(B):
                nc.vector.memset(f_sb[b], 0.0)
            gn_batch(S["e4"], 32, S["st2g"], 2, V_BG2G, V_BG2B, V_BG2C,
                     [S["pfx"][b] for b in range(B)],
                     [f_sb[b][:, :, 1:W + 1] for b in range(B)], AF.Relu,
                     vec_sb, epsT)
            hv_sb = S["hv"]
            for b in range(B):
                nc.vector.tensor_scalar_mul(f_sb[b][:, 0, 1:W + 1], f_sb[b][:, 0, 1:W + 1], hv_sb[:, 0:1])
                nc.vector.tensor_scalar_mul(f_sb[b][:, TH - 1, 1:W + 1], f_sb[b][:, TH - 1, 1:W + 1], hv_sb[:, 1:2])

            s1w_sb = S["wt"][:, WT_S1:WT_S1 + 9, :]
            e1w_sb = S["wt"][:, WT_E1:WT_E1 + 9, :]
            sth = ver.tile([32, B, 2, 2], F32, name="sth", tag="sth")
            sthg = ver.tile([32, B, 2, 2], F32, name="sthg", tag="sthg")
            phx = [[ver.tile([128, TOWN * W], F32, name=f"phx{b}{hd}", tag=f"phx{b}{hd}") for hd in range(2)] for b in range(B)]
            for hd in range(2):
                w_sb = s1w_sb if hd == 0 else e1w_sb
                phd = {}
                for b in range(B):
                    phd[b] = psA.tile([128, TOWN * W], F32, name="mm", tag="mm")
                for tap in range(9):
                    kt, kw = tap // 3, tap % 3
                    for b in range(B):
                        nc.tensor.matmul(phd[b], w_sb[:, tap, :],
                                         f_sb[b][:, kt:kt + TOWN, kw:kw + W],
                                         start=(tap == 0), stop=(tap == 8))
                for b in range(B):
                    stats_from(phd[b], vec_sb[:, V_S1B + hd:V_S1B + hd + 1], S["g4"], 32, sth[:, b, hd, :])
                    nc.vector.tensor_copy(phx[b][hd], phd[b])
            arhi = dram.tile([32, B, 2, 2], F32, name=f"arhi{k}", tag=f"arhi{k}")
            arho = dram.tile([32, B, 2, 2], F32, name=f"arho{k}", tag=f"arho{k}", addr_space="Shared")
            nc.gpsimd.dma_start(out=arhi[:, :, :, :], in_=sth[:, :, :, :])
            if no_cc:
                nc.gpsimd.dma_start(out=arho[:, :, :, :], in_=arhi[:, :, :, :])
            else:
                nc.gpsimd.collective_compute("AllReduce", ALU.add, replica_groups=RG,
                                             ins=[arhi.opt()], outs=[arho.opt()])
            nc.gpsimd.dma_start(out=sthg[:, :, :, :], in_=arho[:, :, :, :])
            S.update(phx=phx, sthg=sthg)

        def stage_T3(S):
            vec_sb, epsT = S["vec"], S["eps"]
            hact = [[ver.tile([128, TOWN * W], BF16, name=f"hact{b}{hd}", tag=f"hact{b}{hd}") for hd in range(2)] for b in range(B)]
            o_t = [[ver.tile([1, TOWN * W], F32, name=f"o{b}{hd}", tag=f"o{b}{hd}") for hd in range(2)] for b in range(B)]
            gn_batch(S["e4"], 32, S["sthg"].rearrange("g b h s -> g (b h) s"), 4,
                     V_BHG, V_BHB, V_BHC,
                     [S["phx"][b][hd] for b in range(B) for hd in range(2)],
                     [hact[b][hd] for b in range(B) for hd in range(2)], AF.Relu,
                     vec_sb, epsT)
            s2w_sb = S["wt"][:, WT_S2, 0:2]
            for b in range(B):
                for hd in range(2):
                    po = psS.tile([1, TOWN * W], F32, name="pst", tag="pst")
                    nc.tensor.matmul(po, s2w_sb[:, hd:hd + 1], hact[b][hd], start=True, stop=True)
                    nc.scalar.activation(out=o_t[b][hd], in_=po, func=AF.Sigmoid,
                                         bias=vec_sb[0:1, V_S2B + hd:V_S2B + hd + 1], scale=1.0)
                    nc.gpsimd.dma_start(out=outd[b, hd], in_=o_t[b][hd])

        states = {}
        for k in range(iters):
            states[k] = stage_F(k)
            if k >= 1:
                stage_T1(states[k - 1])
            if k >= 2:
                stage_T2(states[k - 2])
            if k >= 3:
                stage_T3(states.pop(k - 3))
        # drain
        if iters >= 1:
            stage_T1(states[iters - 1])
        if iters >= 2:
            stage_T2(states[iters - 2])
        if iters >= 3:
            stage_T3(states.pop(iters - 3))
        stage_T2(states[iters - 1])
        if iters >= 2:
            stage_T3(states.pop(iters - 2))
        stage_T3(states.pop(iters - 1))

    nc.compile()
    return nc


_module_cache = {}


def _get_module():
    if "nc" not in _module_cache:
        _module_cache["nc"] = _build()
    return _module_cache["nc"]


def _prep(inputs):
    def f32(a):
        return np.ascontiguousarray(np.asarray(a, dtype=np.float32))

    x = f32(inputs["x"])
    mask = f32(inputs["sample_mask"]).reshape(T, N, T, W)

    c1_w = f32(inputs["c1_w"])
    r3d_w = f32(inputs["r3d_w"])[:, :, :, 0, 0]
    r2d_w = f32(inputs["r2d_w"])[:, :, 0, 0]
    s1_w = f32(inputs["s1_w"])
    e1_w = f32(inputs["e1_w"])
    s2_w = f32(inputs["s2_w"])[0, :, 0, 0]
    e2_w = f32(inputs["e2_w"])[0, :, 0, 0]

    x_h = x.astype(BFNP)

    # conv1 weights: [c, j*4+ct, m] = c1_w[m, ct*128+c, j]
    a = c1_w.transpose(1, 2, 0).reshape(4, 128, 3, H1)
    c1w_h = a.transpose(1, 2, 0, 3).reshape(128, 12, H1).astype(BFNP)

    # r3d weights: [c, n*2+ct, o] = r3d_w[o, ct*128+c, n]
    a = r3d_w.transpose(1, 2, 0).reshape(2, 128, N, H3)
    r3d_h = np.ascontiguousarray(a.transpose(1, 2, 0, 3).reshape(128, 64, H3)).astype(BFNP)

    # packed tail weights [128, 23, 128]
    wtail = np.zeros((128, 23, H2), np.float32)
    wtail[:, WT_R2D:WT_R2D + 4, :] = r2d_w.T.reshape(4, 128, H2).transpose(1, 0, 2)
    wtail[:, WT_S1:WT_S1 + 9, :] = s1_w.transpose(1, 2, 3, 0).reshape(128, 9, H2)
    wtail[:, WT_E1:WT_E1 + 9, :] = e1_w.transpose(1, 2, 3, 0).reshape(128, 9, H2)
    wtail[:, WT_S2, 0] = s2_w
    wtail[:, WT_S2, 1] = e2_w
    wtail_h = wtail.astype(BFNP)

    ch = np.arange(128)
    g8 = (ch[:, None] // 8 == np.arange(16)[None, :]).astype(np.float32)
    g16 = (ch[:, None] // 16 == np.arange(8)[None, :]).astype(np.float32)
    g4 = (ch[:, None] // 4 == np.arange(32)[None, :]).astype(np.float32)
    gmats = np.concatenate([g8 / 8.0, g16 / (16.0 * 8), g4 / (4.0 * 8)], axis=1)
    emats = np.zeros((96, 128), np.float32)
    emats[0:16] = g8.T
    emats[32:40] = g16.T
    emats[64:96] = g4.T

    vecs = np.zeros((NVEC, 128), np.float32)
    vecs[V_C1B:V_C1B + 2] = f32(inputs["c1_b"]).reshape(2, 128)
    vecs[V_GN1G:V_GN1G + 2] = f32(inputs["gn1_g"]).reshape(2, 128)
    vecs[V_GN1B:V_GN1B + 2] = f32(inputs["gn1_b"]).reshape(2, 128)
    vecs[V_R3DB:V_R3DB + 4] = f32(inputs["r3d_b"]).reshape(4, 128)
    vecs[V_GN3G:V_GN3G + 4] = f32(inputs["gn3_g"]).reshape(4, 128)
    vecs[V_GN3B:V_GN3B + 4] = f32(inputs["gn3_b"]).reshape(4, 128)
    vecs[V_R2DB] = f32(inputs["r2d_b"])
    vecs[V_GN2G] = f32(inputs["gn2_g"])
    vecs[V_GN2B] = f32(inputs["gn2_b"])
    vecs[V_S1B] = f32(inputs["s1_b"])
    vecs[V_E1B] = f32(inputs["e1_b"])
    vecs[V_SGNG] = f32(inputs["sgn_g"])
    vecs[V_SGNB] = f32(inputs["sgn_b"])
    vecs[V_EGNG] = f32(inputs["egn_g"])
    vecs[V_EGNB] = f32(inputs["egn_b"])
    vecs[V_S2B] = f32(inputs["s2_b"])[0]
    vecs[V_E2B] = f32(inputs["e2_b"])[0]
    gn1g2 = f32(inputs["gn1_g"]).reshape(2, 128)
    gn1b2 = f32(inputs["gn1_b"]).reshape(2, 128)
    c1b2 = f32(inputs["c1_b"]).reshape(2, 128)
    for i, (b, mt) in enumerate([(b, mt) for b in range(B) for mt in range(2)]):
        vecs[V_BG1G + i] = gn1g2[mt]
        vecs[V_BG1B + i] = gn1b2[mt]
        vecs[V_BG1C + i] = c1b2[mt]
    gn3g4 = f32(inputs["gn3_g"]).reshape(4, 128)
    gn3b4 = f32(inputs["gn3_b"]).reshape(4, 128)
    r3db4 = f32(inputs["r3d_b"]).reshape(4, 128)
    for i, (b, ot) in enumerate([(b, ot) for b in range(B) for ot in range(4)]):
        vecs[V_BG3G + i] = gn3g4[ot]
        vecs[V_BG3B + i] = gn3b4[ot]
        vecs[V_BG3C + i] = r3db4[ot]
    for b in range(B):
        vecs[V_BG2G + b] = f32(inputs["gn2_g"])
        vecs[V_BG2B + b] = f32(inputs["gn2_b"])
        vecs[V_BG2C + b] = f32(inputs["r2d_b"])
    hg = [f32(inputs["sgn_g"]), f32(inputs["egn_g"])]
    hb = [f32(inputs["sgn_b"]), f32(inputs["egn_b"])]
    hc = [f32(inputs["s1_b"]), f32(inputs["e1_b"])]
    for i, (b, hd) in enumerate([(b, hd) for b in range(B) for hd in range(2)]):
        vecs[V_BHG + i] = hg[hd]
        vecs[V_BHB + i] = hb[hd]
        vecs[V_BHC + i] = hc[hd]

    shared = {
        "x_in": x_h, "c1w": c1w_h, "r3dw": r3d_h, "wtail": wtail_h,
        "gmats": gmats, "emats": emats, "vecs": vecs,
    }

    in_maps = []
    for k in range(NCORES):
        t0 = k * TOWN
        tlo = t0 - 1
        m4 = np.zeros((T, N, TH, W), np.float32)
        slo, shi = max(0, tlo), min(T, t0 + TOWN + 1)
        m4[:, :, slo - tlo: shi - tlo, :] = mask[:, :, slo:shi, :]
        # -> [group, partition, ni, tau_tile, col]
        m_h = np.ascontiguousarray(
            m4.reshape(T, N, COLS).transpose(1, 0, 2)       # (N, T, COLS)
              .reshape(NGRP, NG, 2, 128, COLS)
              .transpose(0, 3, 1, 2, 4)                      # (NGRP, 128, NG, 2, COLS)
        ).astype(BFNP)
        hv = np.array([1.0 if k > 0 else 0.0, 1.0 if k < NCORES - 1 else 0.0], np.float32)
        in_maps.append(dict(shared, mask_in=m_h, hv=hv))
    return in_maps


def kernel(**inputs) -> np.ndarray:
    nc = _get_module()
    in_maps = _prep(inputs)
    from concourse.bass_utils import run_bass_kernel_spmd
    res = run_bass_kernel_spmd(nc, in_maps, list(range(NCORES)))
    full = np.zeros((B, 2, T, W), np.float32)
    for k in range(NCORES):
        full[:, :, k * TOWN:(k + 1) * TOWN, :] = res.results[k]["out"]
    return full

